# revision 44
# baseline (speedup 1.0000x reference)
"""Depth-aware 3x3 conv (Wang & Neumann depth-similarity modulated conv) on
8 Trainium2 NeuronCores, batch-parallel (1 image per core).

out[b,o,h,w] = sum_{c,k} weight[o,c,k] * fd[b,k,h,w] * xpatch[b,c,k,h,w] + bias
fd[k,p] = exp(-8.3 * |depth[p + delta_k] - depth[p]|)   (zero-padded patches)

Per-core pipeline (image [64, 256, 256] fp16), per 4096-px chunk:
- x pair tiles via ONE 3-level-AP DMA each:
    T1 = [x ; x@+1]  (pairs (t0,t1) m0=1, (t7,t8) m0=518; center from lower)
    T2 = [x@+3 ; x@+259]  (pairs (t2,t3) m0=0, (t5,t6) m0=258)
- fd gen packed [72 = 9 taps x 8 segs, 512] fp16 (in-place sub/abs/exp on
  DVE+ACT) -> per-chunk DRAM scratch tensor (4 rotating tensors).
- fd replication to 64 rows/tap, split to balance engines:
    pairs 0,1: DMA broadcast ([[FDW,2],[0,64],[1,4096]]), modulated on Pool
    pair 2: 4-row DMA + in-place DVE stream_shuffle quadrant broadcast
    pair 3: one-hot K=2 matmul broadcast into PSUM + ACT evict (PE/ACT
    have slack; DVE does not); pairs 2,3 modulated on DVE.
- modulate mt_g = x2_g o fr_g [128, 4096] in 1024-col pieces so matmul
  groups start before the whole pair is modulated.
- PE: per 512-px group, 5 matmuls (4 pairs K=128 + center K=64) accumulate
  into one PSUM bank; ScalarE evicts with per-partition bias -> fp16.
- Software pipelining: fd-gen runs 3 chunks ahead, fr/x loads 1 chunk ahead.
  SP queue carries only dependency-free loads (x/broadcasts split into
  column halves so first modulate pieces start earlier); the fd scatter and
  out stores ride the ACT queue (fd scatter directly after its exp producer).
Engine busy/chunk (TimelineSim): DMA 14.4us, PE 12.5, DVE 11.9, ACT 11.1,
Pool 10.7; sim total 303us/core vs 593us for the session-start baseline.
"""
import numpy as np

import concourse.bacc as bacc
import concourse.bass as bass
import concourse.mybir as mybir
import concourse.tile as tile
from concourse.bass_utils import run_bass_kernel_spmd

F16 = mybir.dt.float16
F32 = mybir.dt.float32

B, C, H, W = 8, 64, 256, 256
Hp, Wp = H + 2, W + 2          # 258
NP = Hp * Wp                   # 66564
ALPHA = 8.3

CH = 4096                      # output pixels per chunk
NCHUNK = -(-NP // CH)          # 17 (out grid 69632, host slices)
W1T = CH + 520                 # T1 width (halo for m0 up to 518)
W2T = CH + 260                 # T2 width (m0 up to 258)
SEG, SEGW = 8, CH // 8         # fd packing: [72, 512]

# line slacks (elements)
XSL, XSH = 512, 4608           # x line: reads [q0-260, q0+CH+520)
DSL, DSH = 512, 4608           # depth line
XW = XSL + NP + XSH
DW = DSL + NP + DSH
OUTW = NCHUNK * CH             # 69632

FDW = CH                       # fd region width (per parity)

# tap id t = kh*3+kw, delta = (kh-1)*258 + (kw-1)
DELTA = [(kh - 1) * Wp + (kw - 1) for kh in range(3) for kw in range(3)]
# (ta, tb, tile_idx, m0): tb = ta+1; within-pair shift baked in tile upper
PAIRS = [(0, 1, 0, 1), (7, 8, 0, 518), (2, 3, 1, 0), (5, 6, 1, 258)]
# replication method per pair: 'dma' = 64-way broadcast DMA,
# 'shuf' = 4-row DMA + stream_shuffle, 'pe' = one-hot K=2 matmul broadcast
# into PSUM + ACT eviction to SBUF (PE and ACT have slack; DVE does not)
REP = ['dma', 'dma', 'shuf', 'pe']
# modulate engine per pair: DVE cols [0, SPL), Pool cols [SPL, CH).
# dma-pairs go to Pool (their fr is ready early, independent of DVE);
# shuf-pairs go to DVE (shuffle output feeds the same queue, no
# cross-engine serial wait).
MODSPL = [0, 3072, CH, CH]     # columns on DVE


def _build_nc():
    nc = bacc.Bacc("TRN2", target_bir_lowering=False, debug=False, num_devices=8)
    x_line = nc.declare_dram_parameter("x_line", [C, XW], F16, isOutput=False)
    d_line = nc.declare_dram_parameter("d_line", [1, DW], F16, isOutput=False)
    wts = nc.declare_dram_parameter("wts", [128, 7 * 64], F16, isOutput=False)
    bias = nc.declare_dram_parameter("bias", [64, 1], F32, isOutput=False)
    out_l = nc.declare_dram_parameter("out_line", [C, OUTW], F16, isOutput=True)

    x_t = x_line.ap().tensor
    d_t = d_line.ap().tensor
    # One scratch tensor PER parity region: tensor-granularity dependency
    # tracking would otherwise serialize each iteration's fd reads behind
    # the previous iteration's fd write, fusing the DMA streams end-to-start.
    fd_ts = [nc.dram_tensor(f"fd_scratch{r}", [9, FDW], F16).ap().tensor
             for r in range(4)]

    with tile.TileContext(nc) as tc:
        with (
            tc.tile_pool(name="const", bufs=1) as cpool,
            tc.tile_pool(name="xt", bufs=2) as xpool,
            tc.tile_pool(name="fdgen", bufs=2) as gpool,
            tc.tile_pool(name="frep", bufs=2) as fpool,
            tc.tile_pool(name="mmod", bufs=2) as mpool,
            tc.tile_pool(name="ost", bufs=2) as opool,
            tc.tile_pool(name="ps", bufs=6, space="PSUM") as pspool,
            tc.tile_pool(name="psb", bufs=2, space="PSUM") as pbpool,
        ):
            wt_sb = cpool.tile([128, 7 * 64], F16, tag="w")
            nc.sync.dma_start(wt_sb[:], wts[:])
            bias_sb = cpool.tile([64, 1], F32, tag="b")
            nc.sync.dma_start(bias_sb[:], bias[:])

            PIECE = 1024
            NREG = 4  # fd scratch parity regions (write i+3 / read i+1, i)

            def fdgen(i):
                """Depth loads + packed fd gen + scatter to DRAM region i%3.
                Runs TWO iterations ahead of consumption, so its latency
                (sub -> abs -> exp -> scatter) never gates the pipeline."""
                q0 = i * CH
                reg = (i % NREG) * FDW
                dp = gpool.tile([72, SEGW], F16, tag="dp")
                for kh in range(3):
                    nc.sync.dma_start(
                        dp[kh * 24:(kh + 1) * 24, :],
                        bass.AP(d_t, DSL + q0 - 259 + kh * Wp,
                                [[1, 3], [SEGW, SEG], [1, SEGW]]))
                dc = gpool.tile([72, SEGW], F16, tag="dc")
                nc.sync.dma_start(
                    dc[:], bass.AP(d_t, DSL + q0,
                                   [[0, 9], [SEGW, SEG], [1, SEGW]]))
                # sub/abs/exp run in-place on dp (element i is read before
                # it is written within each streaming op) — saves SBUF for
                # the triple-buffered fr pool.
                nc.vector.tensor_tensor(dp[:], dp[:], dc[:],
                                        mybir.AluOpType.subtract)
                nc.scalar.activation(dp[:], dp[:],
                                     mybir.ActivationFunctionType.Abs)
                nc.scalar.activation(dp[:], dp[:],
                                     mybir.ActivationFunctionType.Exp,
                                     scale=-ALPHA)
                return dp

            def fdstore(i, fdp):
                """Scatter packed fd to DRAM scratch tensor i%NREG. Issued
                on the ACT queue right after exp, so its sem-wait never
                blocks the SP queue (whose loads are all dependency-free)."""
                nc.scalar.dma_start(
                    bass.AP(fd_ts[i % NREG], 0,
                            [[FDW, 9], [SEGW, SEG], [1, SEGW]]),
                    fdp[:])

            def frload(i):
                """x1 + fd replication DMAs for chunk i (region written by
                fdgen(i) one iteration earlier). Tiny 4-row loads FIRST (they
                gate the DVE shuffles mid-iteration); x1 and the broadcasts
                are split into column halves and interleaved so the first
                modulate pieces can start ~3us earlier in the DMA stream."""
                q0 = i * CH
                xbase = XSL + q0 - 260
                ft = fd_ts[i % NREG]
                frs = []
                f2 = None
                for g, (ta, tb, ti, m0) in enumerate(PAIRS):
                    fr = fpool.tile([128, CH], F16, tag=f"fr{g}")
                    if REP[g] == 'shuf':
                        nc.sync.dma_start(
                            fr[::32, :],
                            bass.AP(ft, ta * FDW,
                                    [[FDW, 2], [0, 2], [1, CH]]))
                    elif REP[g] == 'pe':
                        f2 = fpool.tile([2, CH], F16, tag="f2")
                        nc.sync.dma_start(
                            f2[:], bass.AP(ft, ta * FDW,
                                           [[FDW, 2], [1, CH]]))
                    frs.append(fr)
                HW1 = W1T // 2
                xt1 = xpool.tile([128, W1T], F16, tag="x1")
                nc.sync.dma_start(
                    xt1[:, 0:HW1],
                    bass.AP(x_t, xbase, [[1, 2], [XW, 64], [1, HW1]]))
                H = CH // 2
                for g, (ta, tb, ti, m0) in enumerate(PAIRS):
                    if REP[g] == 'dma':
                        nc.sync.dma_start(
                            frs[g][:, 0:H],
                            bass.AP(ft, ta * FDW,
                                    [[FDW, 2], [0, 64], [1, H]]))
                nc.sync.dma_start(
                    xt1[:, HW1:W1T],
                    bass.AP(x_t, xbase + HW1, [[1, 2], [XW, 64],
                                               [1, W1T - HW1]]))
                for g, (ta, tb, ti, m0) in enumerate(PAIRS):
                    if REP[g] == 'dma':
                        nc.sync.dma_start(
                            frs[g][:, H:CH],
                            bass.AP(ft, ta * FDW + H,
                                    [[FDW, 2], [0, 64], [1, CH - H]]))
                return [xt1, None], frs, f2

            def xload2(i, tiles):
                q0 = i * CH
                xbase = XSL + q0 - 260
                HW2 = W2T // 2
                xt2 = xpool.tile([128, W2T], F16, tag="x2")
                nc.sync.dma_start(
                    xt2[:, 0:HW2],
                    bass.AP(x_t, xbase + 3, [[256, 2], [XW, 64], [1, HW2]]))
                nc.sync.dma_start(
                    xt2[:, HW2:W2T],
                    bass.AP(x_t, xbase + 3 + HW2,
                            [[256, 2], [XW, 64], [1, W2T - HW2]]))
                tiles[0][1] = xt2

            def shuffles(frs):
                for g in range(4):
                    if REP[g] == 'shuf':
                        nc.vector.stream_shuffle(frs[g][:], frs[g][:],
                                                 [0] * 32)

            def pe_bcast(frs, f2):
                # fd -> [fd_ta x64 ; fd_tb x64] via one-hot K=2 matmul into
                # PSUM, then ACT evicts to the fr SBUF tile in fp16.
                for j in range(CH // 512):
                    psb = pbpool.tile([128, 512], F32)
                    nc.tensor.matmul(
                        psb[:], wt_sb[0:2, 320:448],
                        f2[:, j * 512:(j + 1) * 512], start=True, stop=True)
                    nc.scalar.activation(
                        frs[3][:, j * 512:(j + 1) * 512], psb[:],
                        mybir.ActivationFunctionType.Identity)

            # ---- software-pipelined main loop (3-deep fd skew) ----
            for j in range(3):
                fdstore(j, fdgen(j))
            cur = frload(0)
            xload2(0, cur)
            shuffles(cur[1])
            pe_bcast(cur[1], cur[2])
            for i in range(NCHUNK):
                q0 = i * CH
                # last chunk: only NP - 16*CH = 1028 real columns -> 3 groups
                cw = CH if i + 1 < NCHUNK else 512 * (-(-(NP - q0) // 512))
                xts, frs, _ = cur
                fdp3 = fdgen(i + 3) if i + 3 < NCHUNK else None
                nxt = frload(i + 1) if i + 1 < NCHUNK else None
                if nxt is not None:
                    xload2(i + 1, nxt)
                if fdp3 is not None:
                    fdstore(i + 3, fdp3)

                # modulate in PIECE-col slices, DVE/Pool split per MODSPL
                mts = []
                for g in range(4):
                    mtile = mpool.tile([128, CH], F16, tag=f"m{g}", name=f"m{g}")
                    mts.append(mtile)
                for p0 in range(0, cw, PIECE):
                    pw = min(PIECE, cw - p0)
                    # shuf-pairs (2,3) first: their inputs (own shuffles +
                    # x2) are ready at iteration start; pair1's DVE piece
                    # waits on the fr1 broadcast and would HOL-block them.
                    for g in (2, 3, 0, 1):
                        ta, tb, ti, m0 = PAIRS[g]
                        eng = nc.vector if p0 < MODSPL[g] else nc.gpsimd
                        eng.tensor_tensor(
                            mts[g][:, p0:p0 + pw],
                            xts[ti][:, m0 + p0:m0 + p0 + pw],
                            frs[g][:, p0:p0 + pw], mybir.AluOpType.mult)
                # next chunk's quadrant broadcasts, queued behind our mods
                if nxt is not None:
                    shuffles(nxt[1])

                # ---- matmuls + eviction ----
                ost = opool.tile([64, CH], F16, tag="o")
                for j in range(cw // 512):
                    ps = pspool.tile([64, 512], F32)
                    for g in range(4):
                        nc.tensor.matmul(
                            ps[:], wt_sb[:, g * 64:(g + 1) * 64],
                            mts[g][:, j * 512:(j + 1) * 512],
                            start=(g == 0), stop=False)
                    nc.tensor.matmul(
                        ps[:], wt_sb[0:64, 256:320],
                        xts[0][0:64, 260 + j * 512: 260 + (j + 1) * 512],
                        start=False, stop=True)
                    nc.scalar.activation(
                        ost[:, j * 512:(j + 1) * 512], ps[:],
                        mybir.ActivationFunctionType.Identity,
                        bias=bias_sb[:], scale=1.0)
                # out store on the ACT queue so it can't stall SP-queue loads
                nc.scalar.dma_start(out_l[:, q0:q0 + cw], ost[:, 0:cw])
                # next chunk's PE one-hot fd broadcast (PE queue tail; ACT
                # evicts it after this chunk's out store)
                if nxt is not None:
                    pe_bcast(nxt[1], nxt[2])
                cur = nxt
    nc.compile()
    return nc


_NC_CACHE = None


def _get_nc():
    global _NC_CACHE
    if _NC_CACHE is None:
        _NC_CACHE = _build_nc()
    return _NC_CACHE


def _prep_in_maps(x, depth, weight, bias_np):
    xl = np.zeros((B, C, XW), np.float16)
    xpad = np.zeros((B, C, Hp, Wp), np.float32)
    xpad[:, :, 1:257, 1:257] = x
    xl[:, :, XSL:XSL + NP] = xpad.reshape(B, C, NP).astype(np.float16)

    dl = np.zeros((B, 1, DW), np.float16)
    dpad = np.zeros((B, Hp, Wp), np.float32)
    dpad[:, 1:257, 1:257] = depth[:, 0]
    dl[:, 0, DSL:DSL + NP] = dpad.reshape(B, NP).astype(np.float16)

    wts = np.zeros((128, 7 * 64), np.float16)
    wts[0, 320:384] = 1.0   # one-hot: psum rows 0-63   <- fd row 0 (tap a)
    wts[1, 384:448] = 1.0   # one-hot: psum rows 64-127 <- fd row 1 (tap b)
    for g, (ta, tb, _, _) in enumerate(PAIRS):
        # lhsT[c, o] = weight[o, c, kh, kw]
        wts[0:64, g * 64:(g + 1) * 64] = \
            weight[:, :, ta // 3, ta % 3].T.astype(np.float16)
        wts[64:128, g * 64:(g + 1) * 64] = \
            weight[:, :, tb // 3, tb % 3].T.astype(np.float16)
    wts[0:64, 256:320] = weight[:, :, 1, 1].T.astype(np.float16)

    bias_col = bias_np.reshape(64, 1).astype(np.float32)
    return [
        {"x_line": xl[b], "d_line": dl[b], "wts": wts, "bias": bias_col}
        for b in range(B)
    ]


def kernel(x, depth, weight, bias):
    x = np.asarray(x, dtype=np.float32)
    depth = np.asarray(depth, dtype=np.float32)
    weight = np.asarray(weight, dtype=np.float32)
    bias_np = np.asarray(bias, dtype=np.float32)

    nc = _get_nc()
    in_maps = _prep_in_maps(x, depth, weight, bias_np)
    res = run_bass_kernel_spmd(nc, in_maps, list(range(B)))

    out = np.empty((B, C, H, W), np.float32)
    for b in range(B):
        ol = res.results[b]["out_line"][:, :NP].astype(np.float32)
        out[b] = ol.reshape(C, Hp, Wp)[:, 1:257, 1:257]
    return out
